# revision 18
# baseline (speedup 1.0000x reference)
"""BoxTeacher greedy box-matching + mask gather kernel for Trainium2.

Data-parallel over the batch axis: 8 images -> 8 NeuronCores, one image per
core.  Per core:
  1. pairwise IoU [100,32] with DVE ops (gt coords broadcast across
     partitions with K=1 outer-product matmuls)
  2. active-pred filter (any iou >= 0.5) + descending-score rank compaction
     via a transposed comparison matrix and a one-hot permutation matmul
  3. sequential greedy matching, fully unrolled, wrapped in If-chunks so only
     ~n_active steps execute
  4. biou / flags via one-hot matmuls
  5. mask gather: indirect DMA HBM->SBUF->HBM moving only matched masks;
     unmatched rows use out-of-bounds indices (silently skipped) and the
     pre-zeroed output supplies the zeros.
"""

from contextlib import ExitStack

import numpy as np

import concourse.bass as bass
import concourse.bacc as bacc
import concourse.mybir as mybir
from concourse import tile
from concourse.bass import IndirectOffsetOnAxis

M = 100          # predictions per image
N = 32           # gt boxes per image
B = 8            # batch == cores
HWPROD = 256 * 256
CPM = 8          # chunks per mask
CHUNK = HWPROD // CPM   # 8192 floats per chunk row
VROWS = M * CPM  # 800 source rows
OROWS = N * CPM  # 256 dest rows
THR = 0.5
OOB = 1.0e5  # must stay positive in int32 after *CHUNK (coef) scaling
SENTINEL = 1.4999  # < 1.5 == THR+1; candidate values are 0 or >= 1.5

f32 = mybir.dt.float32
i32 = mybir.dt.int32
Alu = mybir.AluOpType
X = mybir.AxisListType.X
DVE = mybir.EngineType.DVE

USE_IF = True        # wrap scan chunks in If(n_act > c0) to skip dead steps
USE_INDIRECT = True  # gather/scatter via indirect DMA with OOB skipping


def build_nc():
    # Bacc's compile() splits multi-wait matmuls (HW allows one sync wait per
    # PE instruction) — raw Bass has no such pass.
    nc = bacc.Bacc(trn_type="TRN2")

    pb = nc.dram_tensor("pb", [M, 4], f32, kind="ExternalInput")
    gbt = nc.dram_tensor("gbt", [4, N], f32, kind="ExternalInput")
    ps = nc.dram_tensor("ps", [M, 1], f32, kind="ExternalInput")
    psr = nc.dram_tensor("psr", [1, M], f32, kind="ExternalInput")
    ms = nc.dram_tensor("ms", [M, 1], f32, kind="ExternalInput")
    masks = nc.dram_tensor("masks", [VROWS, CHUNK], f32, kind="ExternalInput")

    out_masks = nc.dram_tensor("out_masks", [OROWS, CHUNK], f32, kind="ExternalOutput")
    out_flags = nc.dram_tensor("out_flags", [1, N], f32, kind="ExternalOutput")
    out_gtm = nc.dram_tensor("out_gtm", [1, N], i32, kind="ExternalOutput")
    out_biou = nc.dram_tensor("out_biou", [1, N], f32, kind="ExternalOutput")

    with tile.TileContext(nc) as tc, ExitStack() as ctx:
        sb = ctx.enter_context(tc.tile_pool(name="sb", bufs=1))
        pp = ctx.enter_context(tc.tile_pool(name="pp", bufs=1, space="PSUM"))
        mpool = ctx.enter_context(tc.tile_pool(name="mpool", bufs=2))

        # ---- input loads ----
        pb_t = sb.tile([M, 4], f32)
        nc.sync.dma_start(out=pb_t, in_=pb[:, :])
        ps_t = sb.tile([M, 1], f32)
        nc.sync.dma_start(out=ps_t, in_=ps[:, :])
        psr_t = sb.tile([1, M], f32)
        nc.sync.dma_start(out=psr_t, in_=psr[:, :])
        ms_t = sb.tile([M, 1], f32)
        nc.sync.dma_start(out=ms_t, in_=ms[:, :])
        gbr = []
        for c in range(4):
            t = sb.tile([1, N], f32, name=f"gbr{c}")
            nc.sync.dma_start(out=t, in_=gbt[c : c + 1, :])
            gbr.append(t)

        ones_row = sb.tile([1, 128], f32)
        nc.vector.memset(ones_row, 1.0)
        ones_col = sb.tile([M, 1], f32)
        nc.vector.memset(ones_col, 1.0)

        iota_col_i = sb.tile([M, 1], i32)
        nc.gpsimd.iota(iota_col_i, pattern=[[0, 1]], channel_multiplier=1)
        iota_col = sb.tile([M, 1], f32)
        nc.vector.tensor_copy(out=iota_col, in_=iota_col_i)
        iota_row_i = sb.tile([M, M], i32)
        nc.gpsimd.iota(iota_row_i, pattern=[[1, M]], channel_multiplier=0)
        iota_row = sb.tile([M, M], f32)
        nc.vector.tensor_copy(out=iota_row, in_=iota_row_i)

        def bcast(row_ap, n, ptag, name):
            """Broadcast a [1, n] partition-0 row to all 128 partitions."""
            t = pp.tile([128, n], f32, tag=ptag, name=name)
            nc.tensor.matmul(out=t, lhsT=ones_row, rhs=row_ap, start=True, stop=True)
            return t

        # ---- broadcast gt coords: gbc[c][p, n] = gt_boxes[n, c] ----
        gbc = []
        for c in range(4):
            t = bcast(gbr[c], N, f"tP{c}", f"gbc{c}_ps")
            # ALU ops may read at most one PSUM operand; stage in SBUF.
            s = sb.tile([M, N], f32, name=f"gbs{c}")
            nc.vector.tensor_copy(out=s, in_=t[:M, :])
            gbc.append(s)

        # scores broadcast: sbc[p, j] = scores[j]
        sbc = pp.tile([128, M], f32, tag="tP4")
        nc.tensor.matmul(out=sbc, lhsT=ones_row, rhs=psr_t, start=True, stop=True)

        # ---- IoU [M, N] ----
        px1, py1 = pb_t[:, 0:1], pb_t[:, 1:2]
        px2, py2 = pb_t[:, 2:3], pb_t[:, 3:4]
        MN = [M, N]
        ltx = sb.tile(MN, f32)
        nc.vector.tensor_scalar(out=ltx, in0=gbc[0], scalar1=px1, scalar2=None, op0=Alu.max)
        lty = sb.tile(MN, f32)
        nc.vector.tensor_scalar(out=lty, in0=gbc[1], scalar1=py1, scalar2=None, op0=Alu.max)
        rbx = sb.tile(MN, f32)
        nc.vector.tensor_scalar(out=rbx, in0=gbc[2], scalar1=px2, scalar2=None, op0=Alu.min)
        rby = sb.tile(MN, f32)
        nc.vector.tensor_scalar(out=rby, in0=gbc[3], scalar1=py2, scalar2=None, op0=Alu.min)
        # wx = clip(rbx - ltx, 0), wy likewise
        wx = sb.tile(MN, f32)
        nc.vector.scalar_tensor_tensor(out=wx, in0=ltx, scalar=-1.0, in1=rbx, op0=Alu.mult, op1=Alu.add)
        nc.vector.tensor_scalar(out=wx, in0=wx, scalar1=0.0, scalar2=None, op0=Alu.max)
        wy = sb.tile(MN, f32)
        nc.vector.scalar_tensor_tensor(out=wy, in0=lty, scalar=-1.0, in1=rby, op0=Alu.mult, op1=Alu.add)
        nc.vector.tensor_scalar(out=wy, in0=wy, scalar1=0.0, scalar2=None, op0=Alu.max)
        inter = sb.tile(MN, f32)
        nc.vector.tensor_tensor(out=inter, in0=wx, in1=wy, op=Alu.mult)
        # gt areas broadcast
        agw = sb.tile(MN, f32)
        nc.vector.tensor_tensor(out=agw, in0=gbc[2], in1=gbc[0], op=Alu.subtract)
        agh = sb.tile(MN, f32)
        nc.vector.tensor_tensor(out=agh, in0=gbc[3], in1=gbc[1], op=Alu.subtract)
        nc.vector.tensor_tensor(out=agw, in0=agw, in1=agh, op=Alu.mult)  # agw = gt area
        # pred areas
        apw = sb.tile([M, 1], f32)
        nc.vector.tensor_tensor(out=apw, in0=px2, in1=px1, op=Alu.subtract)
        aph = sb.tile([M, 1], f32)
        nc.vector.tensor_tensor(out=aph, in0=py2, in1=py1, op=Alu.subtract)
        nc.vector.tensor_tensor(out=apw, in0=apw, in1=aph, op=Alu.mult)  # apw = pred area
        union = sb.tile(MN, f32)
        nc.vector.tensor_scalar(out=union, in0=agw, scalar1=apw[:, 0:1], scalar2=None, op0=Alu.add)
        nc.vector.scalar_tensor_tensor(out=union, in0=inter, scalar=-1.0, in1=union, op0=Alu.mult, op1=Alu.add)
        iou_t = sb.tile(MN, f32)
        nc.vector.reciprocal(out=iou_t, in_=union)
        nc.vector.tensor_tensor(out=iou_t, in0=inter, in1=iou_t, op=Alu.mult)

        # ---- active preds + ranks ----
        ok_u = sb.tile(MN, f32)
        nc.vector.tensor_scalar(out=ok_u, in0=iou_t, scalar1=THR, scalar2=None, op0=Alu.is_ge)
        act = sb.tile([M, 1], f32)
        nc.vector.tensor_reduce(out=act, in_=ok_u, axis=X, op=Alu.max)

        # before(j, i) with j on partitions, i on free:
        #   bmat[j, i] = (s[j] > s[i]) + (s[j] == s[i]) * (j < i)
        bmat = sb.tile([M, M], f32)
        nc.vector.tensor_scalar(out=bmat, in0=sbc[:M, :], scalar1=ps_t[:, 0:1], scalar2=None, op0=Alu.is_lt)
        beq = sb.tile([M, M], f32)
        nc.vector.tensor_scalar(out=beq, in0=sbc[:M, :], scalar1=ps_t[:, 0:1], scalar2=None, op0=Alu.is_equal)
        jgt = sb.tile([M, M], f32)  # [j, i] = (i > j)
        nc.vector.tensor_scalar(out=jgt, in0=iota_row, scalar1=iota_col[:, 0:1], scalar2=None, op0=Alu.is_gt)
        nc.vector.tensor_tensor(out=beq, in0=beq, in1=jgt, op=Alu.mult)
        nc.vector.tensor_tensor(out=bmat, in0=bmat, in1=beq, op=Alu.add)
        # rank among active: ranka[i] = sum_j bmat[j, i] * act[j]
        bact = sb.tile([M, M], f32)
        nc.vector.tensor_scalar(out=bact, in0=bmat, scalar1=act[:, 0:1], scalar2=None, op0=Alu.mult)
        ranka_ps = pp.tile([M, 1], f32, tag="tP5")
        nc.tensor.matmul(out=ranka_ps, lhsT=bact, rhs=ones_col[:, 0:1], start=True, stop=True)
        ranka = sb.tile([M, 1], f32)
        nc.vector.tensor_copy(out=ranka, in_=ranka_ps)
        pen = sb.tile([M, 1], f32)
        nc.vector.tensor_scalar(out=pen, in0=act, scalar1=-200.0, scalar2=200.0, op0=Alu.mult, op1=Alu.add)
        nc.vector.tensor_tensor(out=ranka, in0=ranka, in1=pen, op=Alu.add)  # rprime
        onehot = sb.tile([M, M], f32)
        nc.vector.tensor_scalar(out=onehot, in0=iota_row, scalar1=ranka[:, 0:1], scalar2=None, op0=Alu.is_equal)

        # n_active -> int32 scalar for the If cascade
        nact_ps = pp.tile([1, 1], f32, tag="tP2x")
        nc.tensor.matmul(out=nact_ps, lhsT=act[:, 0:1], rhs=ones_col[:, 0:1], start=True, stop=True)
        nact_i = sb.tile([1, 1], i32)
        nc.vector.tensor_copy(out=nact_i, in_=nact_ps)

        # sorted iou rows (score-descending among active), G = (iou+1)*(iou>=thr)
        ious = pp.tile([M, N], f32, tag="tP0")
        nc.tensor.matmul(out=ious, lhsT=onehot, rhs=iou_t, start=True, stop=True)
        okm = sb.tile([M, N], i32)  # copy_predicated masks must be integer dtype
        nc.vector.tensor_scalar(out=okm, in0=ious, scalar1=THR, scalar2=None, op0=Alu.is_ge)
        iop1 = sb.tile([M, N], f32)
        nc.vector.tensor_scalar(out=iop1, in0=ious, scalar1=1.0, scalar2=None, op0=Alu.add)
        G = sb.tile([M, N], f32)
        nc.vector.memset(G, 0.0)
        nc.vector.copy_predicated(out=G, mask=okm, data=iop1)  # NaN-safe

        orig_ps = pp.tile([1, M], f32, tag="tP1")
        nc.tensor.matmul(out=orig_ps, lhsT=iota_col[:, 0:1], rhs=onehot, start=True, stop=True)
        orig_row = sb.tile([1, M], f32)
        nc.vector.tensor_copy(out=orig_row, in_=orig_ps)

        # DVE operands need partition-aligned bases, so flatten G's rows onto
        # partition 0 for the scan's per-step row slices.
        g_flat = sb.tile([1, M * N], f32)
        nc.sync.dma_start(
            out=g_flat[0:1, :].rearrange("p (k n) -> p k n", n=N), in_=G[:, :]
        )

        # ---- greedy scan ----
        # skip_runtime_bounds_check: the emitted runtime assert aborts NEFF
        # execution under the PJRT path; nact is structurally within [0, M].
        nact_v = nc.values_load(
            nact_i[0:1, 0:1], engines=(DVE,), min_val=0, max_val=M,
            skip_runtime_bounds_check=True,
        )
        gtm_r = sb.tile([1, N], f32)
        nc.vector.memset(gtm_r, -1.0)
        cp1 = sb.tile([1, N], f32)
        mx = sb.tile([1, 1], f32)
        mxc = sb.tile([1, 1], f32)
        sel = sb.tile([1, N], i32)

        from contextlib import nullcontext

        CHS = 8
        for c0 in range(0, M, CHS):
            with tc.If(nact_v > c0) if USE_IF else nullcontext():
                for k in range(c0, min(c0 + CHS, M)):
                    # cand+1 = (gtm <= 0) * G[k]
                    nc.vector.scalar_tensor_tensor(
                        out=cp1, in0=gtm_r, scalar=0.0,
                        in1=g_flat[0:1, k * N : (k + 1) * N],
                        op0=Alu.is_le, op1=Alu.mult,
                    )
                    nc.vector.tensor_reduce(out=mx, in_=cp1, axis=X, op=Alu.max)
                    nc.vector.tensor_scalar(out=mxc, in0=mx, scalar1=SENTINEL, scalar2=None, op0=Alu.max)
                    nc.vector.tensor_scalar(out=sel, in0=cp1, scalar1=mxc[0:1, 0:1], scalar2=None, op0=Alu.is_equal)
                    nc.vector.copy_predicated(
                        out=gtm_r, mask=sel,
                        data=orig_row[0:1, k : k + 1].to_broadcast([1, N]),
                    )

        # ---- outputs: gtm, matched, biou, flags ----
        gtm_i = sb.tile([1, N], i32)
        nc.vector.tensor_copy(out=gtm_i, in_=gtm_r)
        nc.sync.dma_start(out=out_gtm[:, :], in_=gtm_i)

        matched = sb.tile([1, N], f32)
        nc.vector.tensor_scalar(out=matched, in0=gtm_r, scalar1=0.0, scalar2=None, op0=Alu.is_ge)

        gtm_bc = bcast(gtm_r, N, "tP3", "gtm_bc")
        onehot2 = sb.tile([M, N], f32)
        nc.vector.tensor_scalar(out=onehot2, in0=gtm_bc[:M, :], scalar1=iota_col[:, 0:1], scalar2=None, op0=Alu.is_equal)
        onehot2_i = sb.tile([M, N], i32)
        nc.vector.tensor_scalar(out=onehot2_i, in0=gtm_bc[:M, :], scalar1=iota_col[:, 0:1], scalar2=None, op0=Alu.is_equal)
        miou = sb.tile([M, N], f32)
        nc.vector.memset(miou, 0.0)
        nc.vector.copy_predicated(out=miou, mask=onehot2_i, data=iou_t)  # NaN-safe
        biou_ps = pp.tile([1, N], f32, tag="tP4")
        nc.tensor.matmul(out=biou_ps, lhsT=ones_col[:, 0:1], rhs=miou, start=True, stop=True)
        flags_ps = pp.tile([1, N], f32, tag="tP5")
        nc.tensor.matmul(out=flags_ps, lhsT=ms_t[:, 0:1], rhs=onehot2, start=True, stop=True)
        biou_s = sb.tile([1, N], f32)
        nc.vector.tensor_copy(out=biou_s, in_=biou_ps)
        nc.sync.dma_start(out=out_biou[:, :], in_=biou_s)
        flags_s = sb.tile([1, N], f32)
        nc.vector.tensor_copy(out=flags_s, in_=flags_ps)
        nc.sync.dma_start(out=out_flags[:, :], in_=flags_s)

        # ---- mask gather ----
        iotap_i = sb.tile([128, 1], i32)
        nc.gpsimd.iota(iotap_i, pattern=[[0, 1]], channel_multiplier=1)
        pm8_i = sb.tile([128, 1], i32)
        nc.vector.tensor_scalar(out=pm8_i, in0=iotap_i, scalar1=7, scalar2=None, op0=Alu.bitwise_and)
        pm8_f = sb.tile([128, 1], f32)
        nc.vector.tensor_copy(out=pm8_f, in_=pm8_i)
        pd8_i = sb.tile([128, 1], i32)
        nc.vector.tensor_scalar(out=pd8_i, in0=iotap_i, scalar1=3, scalar2=None, op0=Alu.arith_shift_right)
        pd8_f = sb.tile([128, 1], f32)
        nc.vector.tensor_copy(out=pd8_f, in_=pd8_i)
        pidx_f = sb.tile([128, 1], f32)
        nc.vector.tensor_copy(out=pidx_f, in_=iotap_i)
        iota_n_i = sb.tile([128, N], i32)
        nc.gpsimd.iota(iota_n_i, pattern=[[1, N]], channel_multiplier=0)
        iota_n = sb.tile([128, N], f32)
        nc.vector.tensor_copy(out=iota_n, in_=iota_n_i)

        # srow[n] = matched ? gtm[n]*CPM : huge
        srow = sb.tile([1, N], f32)
        nc.vector.tensor_scalar(out=srow, in0=matched, scalar1=-OOB, scalar2=OOB, op0=Alu.mult, op1=Alu.add)
        nc.vector.scalar_tensor_tensor(out=srow, in0=gtm_r, scalar=float(CPM), in1=srow, op0=Alu.mult, op1=Alu.add)
        srow_bc = bcast(srow, N, "tP1", "srow_bc")

        for h in range(2):
            nofp = sb.tile([128, 1], f32, tag="nofp")
            nc.vector.tensor_scalar(out=nofp, in0=pd8_f, scalar1=float(16 * h), scalar2=None, op0=Alu.add)
            sel8 = sb.tile([128, N], f32, tag="sel8")
            nc.vector.tensor_scalar(out=sel8, in0=iota_n, scalar1=nofp[:, 0:1], scalar2=None, op0=Alu.is_equal)
            nc.vector.tensor_tensor(out=sel8, in0=sel8, in1=srow_bc[:, :], op=Alu.mult)
            gsum = sb.tile([128, 1], f32, tag="gsum")
            nc.vector.tensor_reduce(out=gsum, in_=sel8, axis=X, op=Alu.add)
            nc.vector.tensor_tensor(out=gsum, in0=gsum, in1=pm8_f, op=Alu.add)
            tky = sb.tile([128, 1], f32, tag="tky")
            nc.vector.tensor_scalar(out=tky, in0=gsum, scalar1=float(VROWS), scalar2=None, op0=Alu.is_lt)
            mt = mpool.tile([128, CHUNK], f32, tag="mt")
            if USE_INDIRECT:
                gidx_i = sb.tile([128, 1], i32, tag="gidx")
                nc.vector.tensor_copy(out=gidx_i, in_=gsum)
                nc.gpsimd.indirect_dma_start(
                    out=mt[:, :], out_offset=None,
                    in_=masks[:, :],
                    in_offset=IndirectOffsetOnAxis(ap=gidx_i[:, 0:1], axis=0),
                    bounds_check=VROWS - 1, oob_is_err=False,
                )
                dd = sb.tile([128, 1], f32, tag="dd")
                nc.vector.tensor_scalar(out=dd, in0=pidx_f, scalar1=float(128 * h) - OOB, scalar2=None, op0=Alu.add)
                nc.vector.tensor_tensor(out=dd, in0=dd, in1=tky, op=Alu.mult)
                nc.vector.tensor_scalar(out=dd, in0=dd, scalar1=OOB, scalar2=None, op0=Alu.add)
                didx_i = sb.tile([128, 1], i32, tag="didx")
                nc.vector.tensor_copy(out=didx_i, in_=dd)
                nc.gpsimd.indirect_dma_start(
                    out=out_masks[:, :],
                    out_offset=IndirectOffsetOnAxis(ap=didx_i[:, 0:1], axis=0),
                    in_=mt[:, :], in_offset=None,
                    bounds_check=OROWS - 1, oob_is_err=False,
                )
            else:
                # bisect fallback: clamped in-bounds gather, zero unmatched
                # rows on-chip, store all rows.
                gclip = sb.tile([128, 1], f32, tag="gclip")
                nc.vector.tensor_scalar(out=gclip, in0=gsum, scalar1=float(VROWS - 1), scalar2=None, op0=Alu.min)
                gidx_i = sb.tile([128, 1], i32, tag="gidx")
                nc.vector.tensor_copy(out=gidx_i, in_=gclip)
                nc.gpsimd.indirect_dma_start(
                    out=mt[:, :], out_offset=None,
                    in_=masks[:, :],
                    in_offset=IndirectOffsetOnAxis(ap=gidx_i[:, 0:1], axis=0),
                )
                nc.vector.tensor_scalar(out=mt, in0=mt, scalar1=tky[:, 0:1], scalar2=None, op0=Alu.mult)
                nc.sync.dma_start(
                    out=out_masks[128 * h : 128 * (h + 1), :], in_=mt[:, :]
                )

    nc.compile()
    return nc


_NC = None


def _get_nc():
    global _NC
    if _NC is None:
        _NC = build_nc()
    return _NC


def make_in_maps(pred_boxes, gt_boxes, pred_scores, pred_masks, mask_score):
    pred_boxes = np.asarray(pred_boxes, np.float32)
    gt_boxes = np.asarray(gt_boxes, np.float32)
    pred_scores = np.asarray(pred_scores, np.float32)
    pred_masks = np.asarray(pred_masks, np.float32)
    mask_score = np.asarray(mask_score, np.float32)
    in_maps = []
    for b in range(B):
        in_maps.append(
            {
                "pb": np.ascontiguousarray(pred_boxes[b]),
                "gbt": np.ascontiguousarray(gt_boxes[b].T),
                "ps": np.ascontiguousarray(pred_scores[b].reshape(M, 1)),
                "psr": np.ascontiguousarray(pred_scores[b].reshape(1, M)),
                "ms": np.ascontiguousarray(mask_score[b].reshape(M, 1)),
                "masks": np.ascontiguousarray(pred_masks[b].reshape(VROWS, CHUNK)),
            }
        )
    return in_maps


def unpack_outs(outs):
    gt_masks = np.stack([outs[b]["out_masks"].reshape(N, 256, 256) for b in range(B)])
    flags = np.stack([outs[b]["out_flags"].reshape(N) for b in range(B)])
    gtm = np.stack([outs[b]["out_gtm"].reshape(N).astype(np.int32) for b in range(B)])
    biou = np.stack([outs[b]["out_biou"].reshape(N) for b in range(B)])
    return gt_masks, flags, gtm, biou


def kernel(pred_boxes, gt_boxes, pred_scores, pred_masks, mask_score):
    from concourse.bass_utils import run_bass_kernel_spmd

    in_maps = make_in_maps(pred_boxes, gt_boxes, pred_scores, pred_masks, mask_score)
    res = run_bass_kernel_spmd(_get_nc(), in_maps, core_ids=list(range(B)))
    return unpack_outs(res.results)


# revision 24
# speedup vs baseline: 1.0811x; 1.0811x over previous
"""BoxTeacher greedy box-matching + mask gather kernel for Trainium2.

Data-parallel over the batch axis: 8 images -> 8 NeuronCores, one image per
core.  Per core:
  1. pairwise IoU [100,32] with DVE ops (gt coords broadcast across
     partitions with K=1 outer-product matmuls)
  2. active-pred filter (any iou >= 0.5) + descending-score rank compaction
     via a transposed comparison matrix and a one-hot permutation matmul
  3. sequential greedy matching, fully unrolled, wrapped in If-chunks so only
     ~n_active steps execute
  4. biou / flags via one-hot matmuls
  5. mask gather: indirect DMA HBM->SBUF->HBM moving only matched masks;
     unmatched rows use out-of-bounds indices (silently skipped) and the
     pre-zeroed output supplies the zeros.
"""

from contextlib import ExitStack

import numpy as np

import concourse.bass as bass
import concourse.bacc as bacc
import concourse.mybir as mybir
from concourse import tile
from concourse.bass import IndirectOffsetOnAxis

M = 100          # predictions per image
N = 32           # gt boxes per image
B = 8            # batch == cores
HWPROD = 256 * 256
CPM = 4          # chunks per mask
CHUNK = HWPROD // CPM   # 16384 floats per chunk row
VROWS = M * CPM  # 400 source rows
OROWS = N * CPM  # 128 dest rows
THR = 0.5
OOB = 1.0e5  # must stay positive in int32 after *CHUNK (coef) scaling
SENTINEL = 1.4999  # < 1.5 == THR+1; candidate values are 0 or >= 1.5

f32 = mybir.dt.float32
i32 = mybir.dt.int32
Alu = mybir.AluOpType
X = mybir.AxisListType.X
DVE = mybir.EngineType.DVE

USE_IF = True        # wrap scan chunks in If(n_act > c0) to skip dead steps
USE_INDIRECT = True  # gather/scatter via indirect DMA with OOB skipping
UNROLL = 16          # statically unrolled scan steps; the rest run in For_i
CHS = 4              # If-guard granularity within the unrolled steps


def build_nc():
    # Bacc's compile() splits multi-wait matmuls (HW allows one sync wait per
    # PE instruction) — raw Bass has no such pass.
    nc = bacc.Bacc(trn_type="TRN2")

    pb = nc.dram_tensor("pb", [M, 4], f32, kind="ExternalInput")
    gbt = nc.dram_tensor("gbt", [4, N], f32, kind="ExternalInput")
    ps = nc.dram_tensor("ps", [M, 1], f32, kind="ExternalInput")
    psr = nc.dram_tensor("psr", [1, M], f32, kind="ExternalInput")
    ms = nc.dram_tensor("ms", [M, 1], f32, kind="ExternalInput")
    masks = nc.dram_tensor("masks", [VROWS, CHUNK], f32, kind="ExternalInput")

    out_masks = nc.dram_tensor("out_masks", [OROWS, CHUNK], f32, kind="ExternalOutput")
    out_flags = nc.dram_tensor("out_flags", [1, N], f32, kind="ExternalOutput")
    out_gtm = nc.dram_tensor("out_gtm", [1, N], i32, kind="ExternalOutput")
    out_biou = nc.dram_tensor("out_biou", [1, N], f32, kind="ExternalOutput")

    with tile.TileContext(nc) as tc, ExitStack() as ctx:
        sb = ctx.enter_context(tc.tile_pool(name="sb", bufs=1))
        pp = ctx.enter_context(tc.tile_pool(name="pp", bufs=1, space="PSUM"))
        mpool = ctx.enter_context(tc.tile_pool(name="mpool", bufs=2))

        # ---- input loads ----
        pb_t = sb.tile([M, 4], f32)
        nc.sync.dma_start(out=pb_t, in_=pb[:, :])
        ps_t = sb.tile([M, 1], f32)
        nc.sync.dma_start(out=ps_t, in_=ps[:, :])
        psr_t = sb.tile([1, M], f32)
        nc.sync.dma_start(out=psr_t, in_=psr[:, :])
        ms_t = sb.tile([M, 1], f32)
        nc.sync.dma_start(out=ms_t, in_=ms[:, :])
        gbr = []
        for c in range(4):
            t = sb.tile([1, N], f32, name=f"gbr{c}")
            nc.sync.dma_start(out=t, in_=gbt[c : c + 1, :])
            gbr.append(t)

        ones_row = sb.tile([1, 128], f32)
        nc.vector.memset(ones_row, 1.0)
        ones_col = sb.tile([M, 1], f32)
        nc.vector.memset(ones_col, 1.0)

        iota_col_i = sb.tile([M, 1], i32)
        nc.gpsimd.iota(iota_col_i, pattern=[[0, 1]], channel_multiplier=1)
        iota_col = sb.tile([M, 1], f32)
        nc.vector.tensor_copy(out=iota_col, in_=iota_col_i)
        iota_row_i = sb.tile([M, M], i32)
        nc.gpsimd.iota(iota_row_i, pattern=[[1, M]], channel_multiplier=0)
        iota_row = sb.tile([M, M], f32)
        nc.vector.tensor_copy(out=iota_row, in_=iota_row_i)

        def bcast(row_ap, n, ptag, name):
            """Broadcast a [1, n] partition-0 row to all 128 partitions."""
            t = pp.tile([128, n], f32, tag=ptag, name=name)
            nc.tensor.matmul(out=t, lhsT=ones_row, rhs=row_ap, start=True, stop=True)
            return t

        # ---- broadcast gt coords: gbc[c][p, n] = gt_boxes[n, c] ----
        gbc = []
        for c in range(4):
            t = bcast(gbr[c], N, f"tP{c}", f"gbc{c}_ps")
            # ALU ops may read at most one PSUM operand; stage in SBUF.
            s = sb.tile([M, N], f32, name=f"gbs{c}")
            nc.vector.tensor_copy(out=s, in_=t[:M, :])
            gbc.append(s)

        # scores broadcast: sbc[p, j] = scores[j]
        sbc = pp.tile([128, M], f32, tag="tP4")
        nc.tensor.matmul(out=sbc, lhsT=ones_row, rhs=psr_t, start=True, stop=True)

        # ---- IoU [M, N] ----
        px1, py1 = pb_t[:, 0:1], pb_t[:, 1:2]
        px2, py2 = pb_t[:, 2:3], pb_t[:, 3:4]
        MN = [M, N]
        ltx = sb.tile(MN, f32)
        nc.vector.tensor_scalar(out=ltx, in0=gbc[0], scalar1=px1, scalar2=None, op0=Alu.max)
        lty = sb.tile(MN, f32)
        nc.vector.tensor_scalar(out=lty, in0=gbc[1], scalar1=py1, scalar2=None, op0=Alu.max)
        rbx = sb.tile(MN, f32)
        nc.vector.tensor_scalar(out=rbx, in0=gbc[2], scalar1=px2, scalar2=None, op0=Alu.min)
        rby = sb.tile(MN, f32)
        nc.vector.tensor_scalar(out=rby, in0=gbc[3], scalar1=py2, scalar2=None, op0=Alu.min)
        # wx = clip(rbx - ltx, 0), wy likewise
        wx = sb.tile(MN, f32)
        nc.vector.scalar_tensor_tensor(out=wx, in0=ltx, scalar=-1.0, in1=rbx, op0=Alu.mult, op1=Alu.add)
        nc.vector.tensor_scalar(out=wx, in0=wx, scalar1=0.0, scalar2=None, op0=Alu.max)
        wy = sb.tile(MN, f32)
        nc.vector.scalar_tensor_tensor(out=wy, in0=lty, scalar=-1.0, in1=rby, op0=Alu.mult, op1=Alu.add)
        nc.vector.tensor_scalar(out=wy, in0=wy, scalar1=0.0, scalar2=None, op0=Alu.max)
        inter = sb.tile(MN, f32)
        nc.vector.tensor_tensor(out=inter, in0=wx, in1=wy, op=Alu.mult)
        # gt areas broadcast
        agw = sb.tile(MN, f32)
        nc.vector.tensor_tensor(out=agw, in0=gbc[2], in1=gbc[0], op=Alu.subtract)
        agh = sb.tile(MN, f32)
        nc.vector.tensor_tensor(out=agh, in0=gbc[3], in1=gbc[1], op=Alu.subtract)
        nc.vector.tensor_tensor(out=agw, in0=agw, in1=agh, op=Alu.mult)  # agw = gt area
        # pred areas
        apw = sb.tile([M, 1], f32)
        nc.vector.tensor_tensor(out=apw, in0=px2, in1=px1, op=Alu.subtract)
        aph = sb.tile([M, 1], f32)
        nc.vector.tensor_tensor(out=aph, in0=py2, in1=py1, op=Alu.subtract)
        nc.vector.tensor_tensor(out=apw, in0=apw, in1=aph, op=Alu.mult)  # apw = pred area
        union = sb.tile(MN, f32)
        nc.vector.tensor_scalar(out=union, in0=agw, scalar1=apw[:, 0:1], scalar2=None, op0=Alu.add)
        nc.vector.scalar_tensor_tensor(out=union, in0=inter, scalar=-1.0, in1=union, op0=Alu.mult, op1=Alu.add)
        iou_t = sb.tile(MN, f32)
        nc.vector.reciprocal(out=iou_t, in_=union)
        nc.vector.tensor_tensor(out=iou_t, in0=inter, in1=iou_t, op=Alu.mult)

        # ---- active preds + ranks ----
        ok_u = sb.tile(MN, f32)
        nc.vector.tensor_scalar(out=ok_u, in0=iou_t, scalar1=THR, scalar2=None, op0=Alu.is_ge)
        act = sb.tile([M, 1], f32)
        nc.vector.tensor_reduce(out=act, in_=ok_u, axis=X, op=Alu.max)

        # before(j, i) with j on partitions, i on free:
        #   bmat[j, i] = (s[j] > s[i]) + (s[j] == s[i]) * (j < i)
        bmat = sb.tile([M, M], f32)
        nc.vector.tensor_scalar(out=bmat, in0=sbc[:M, :], scalar1=ps_t[:, 0:1], scalar2=None, op0=Alu.is_lt)
        beq = sb.tile([M, M], f32)
        nc.vector.tensor_scalar(out=beq, in0=sbc[:M, :], scalar1=ps_t[:, 0:1], scalar2=None, op0=Alu.is_equal)
        jgt = sb.tile([M, M], f32)  # [j, i] = (i > j)
        nc.vector.tensor_scalar(out=jgt, in0=iota_row, scalar1=iota_col[:, 0:1], scalar2=None, op0=Alu.is_gt)
        nc.vector.tensor_tensor(out=beq, in0=beq, in1=jgt, op=Alu.mult)
        nc.vector.tensor_tensor(out=bmat, in0=bmat, in1=beq, op=Alu.add)
        # rank among active: ranka[i] = sum_j bmat[j, i] * act[j]
        bact = sb.tile([M, M], f32)
        nc.vector.tensor_scalar(out=bact, in0=bmat, scalar1=act[:, 0:1], scalar2=None, op0=Alu.mult)
        ranka_ps = pp.tile([M, 1], f32, tag="tP5")
        nc.tensor.matmul(out=ranka_ps, lhsT=bact, rhs=ones_col[:, 0:1], start=True, stop=True)
        ranka = sb.tile([M, 1], f32)
        nc.vector.tensor_copy(out=ranka, in_=ranka_ps)
        pen = sb.tile([M, 1], f32)
        nc.vector.tensor_scalar(out=pen, in0=act, scalar1=-200.0, scalar2=200.0, op0=Alu.mult, op1=Alu.add)
        nc.vector.tensor_tensor(out=ranka, in0=ranka, in1=pen, op=Alu.add)  # rprime
        onehot = sb.tile([M, M], f32)
        nc.vector.tensor_scalar(out=onehot, in0=iota_row, scalar1=ranka[:, 0:1], scalar2=None, op0=Alu.is_equal)

        # n_active -> int32 scalar for the If cascade
        nact_ps = pp.tile([1, 1], f32, tag="tP2x")
        nc.tensor.matmul(out=nact_ps, lhsT=act[:, 0:1], rhs=ones_col[:, 0:1], start=True, stop=True)
        nact_i = sb.tile([1, 1], i32)
        nc.vector.tensor_copy(out=nact_i, in_=nact_ps)

        # sorted iou rows (score-descending among active), G = (iou+1)*(iou>=thr)
        ious = pp.tile([M, N], f32, tag="tP0")
        nc.tensor.matmul(out=ious, lhsT=onehot, rhs=iou_t, start=True, stop=True)
        okm = sb.tile([M, N], i32)  # copy_predicated masks must be integer dtype
        nc.vector.tensor_scalar(out=okm, in0=ious, scalar1=THR, scalar2=None, op0=Alu.is_ge)
        iop1 = sb.tile([M, N], f32)
        nc.vector.tensor_scalar(out=iop1, in0=ious, scalar1=1.0, scalar2=None, op0=Alu.add)
        G = sb.tile([M, N], f32)
        nc.vector.memset(G, 0.0)
        nc.vector.copy_predicated(out=G, mask=okm, data=iop1)  # NaN-safe

        orig_ps = pp.tile([1, M], f32, tag="tP1")
        nc.tensor.matmul(out=orig_ps, lhsT=iota_col[:, 0:1], rhs=onehot, start=True, stop=True)
        orig_row = sb.tile([1, M], f32)
        nc.vector.tensor_copy(out=orig_row, in_=orig_ps)

        # DVE operands need partition-aligned bases, so flatten G's rows onto
        # partition 0 for the scan's per-step row slices.  Split the flatten
        # so the first UNROLL steps can start before the long tail lands.
        g_flat = sb.tile([1, M * N], f32)
        nc.sync.dma_start(
            out=g_flat[0:1, : UNROLL * N].rearrange("p (k n) -> p k n", n=N),
            in_=G[:UNROLL, :],
        )
        nc.sync.dma_start(
            out=g_flat[0:1, UNROLL * N :].rearrange("p (k n) -> p k n", n=N),
            in_=G[UNROLL:, :],
        )

        # ---- greedy scan ----
        # skip_runtime_bounds_check: the emitted runtime assert aborts NEFF
        # execution under the PJRT path; nact is structurally within [0, M].
        nact_v = nc.values_load(
            nact_i[0:1, 0:1], engines=(DVE,), min_val=0, max_val=M,
            skip_runtime_bounds_check=True,
        )
        # For_i's back-edge barrier needs the bound on every engine.
        nact_all = nc.values_load(
            nact_i[0:1, 0:1], min_val=0, max_val=M,
            skip_runtime_bounds_check=True,
        )
        gtm_r = sb.tile([1, N], f32)
        nc.vector.memset(gtm_r, -1.0)
        cp1 = sb.tile([1, N], f32)
        mx = sb.tile([1, 1], f32)
        mxc = sb.tile([1, 1], f32)
        sel = sb.tile([1, N], i32)

        def scan_step(g_row, ik_cell):
            # cand+1 = (gtm <= 0) * G[k]
            nc.vector.scalar_tensor_tensor(
                out=cp1, in0=gtm_r, scalar=0.0, in1=g_row,
                op0=Alu.is_le, op1=Alu.mult,
            )
            nc.vector.tensor_reduce(out=mx, in_=cp1, axis=X, op=Alu.max)
            nc.vector.tensor_scalar(out=mxc, in0=mx, scalar1=SENTINEL, scalar2=None, op0=Alu.max)
            nc.vector.tensor_scalar(out=sel, in0=cp1, scalar1=mxc[0:1, 0:1], scalar2=None, op0=Alu.is_equal)
            nc.vector.copy_predicated(out=gtm_r, mask=sel, data=ik_cell.to_broadcast([1, N]))

        from contextlib import nullcontext

        # Fast path: UNROLL static steps guarded at CHS granularity (near
        # branches inside one IRAM block).  Rare overflow (n_act > UNROLL)
        # runs in a dynamic loop — slow per step, but typically 0 iterations.
        for c0 in range(0, UNROLL, CHS):
            with tc.If(nact_v > c0) if USE_IF else nullcontext():
                for k in range(c0, c0 + CHS):
                    scan_step(
                        g_flat[0:1, k * N : (k + 1) * N],
                        orig_row[0:1, k : k + 1],
                    )
        with tc.For_i(UNROLL, nact_all, 1) as kv:
            scan_step(
                g_flat[0:1, bass.ds(kv * N, N)],
                orig_row[0:1, bass.ds(kv, 1)],
            )

        # ---- outputs: gtm, matched, biou, flags ----
        gtm_i = sb.tile([1, N], i32)
        nc.vector.tensor_copy(out=gtm_i, in_=gtm_r)
        nc.sync.dma_start(out=out_gtm[:, :], in_=gtm_i)

        matched = sb.tile([1, N], f32)
        nc.vector.tensor_scalar(out=matched, in0=gtm_r, scalar1=0.0, scalar2=None, op0=Alu.is_ge)

        gtm_bc = bcast(gtm_r, N, "tP3", "gtm_bc")
        onehot2 = sb.tile([M, N], f32)
        nc.vector.tensor_scalar(out=onehot2, in0=gtm_bc[:M, :], scalar1=iota_col[:, 0:1], scalar2=None, op0=Alu.is_equal)
        onehot2_i = sb.tile([M, N], i32)
        nc.vector.tensor_scalar(out=onehot2_i, in0=gtm_bc[:M, :], scalar1=iota_col[:, 0:1], scalar2=None, op0=Alu.is_equal)
        miou = sb.tile([M, N], f32)
        nc.vector.memset(miou, 0.0)
        nc.vector.copy_predicated(out=miou, mask=onehot2_i, data=iou_t)  # NaN-safe
        biou_ps = pp.tile([1, N], f32, tag="tP4")
        nc.tensor.matmul(out=biou_ps, lhsT=ones_col[:, 0:1], rhs=miou, start=True, stop=True)
        flags_ps = pp.tile([1, N], f32, tag="tP5")
        nc.tensor.matmul(out=flags_ps, lhsT=ms_t[:, 0:1], rhs=onehot2, start=True, stop=True)
        biou_s = sb.tile([1, N], f32)
        nc.vector.tensor_copy(out=biou_s, in_=biou_ps)
        nc.sync.dma_start(out=out_biou[:, :], in_=biou_s)
        flags_s = sb.tile([1, N], f32)
        nc.vector.tensor_copy(out=flags_s, in_=flags_ps)
        nc.sync.dma_start(out=out_flags[:, :], in_=flags_s)

        # ---- mask gather: one [128, CHUNK] indirect gather + one scatter ----
        # Partition p carries chunk (p & 3) of output slot n = p >> 2.
        iotap_i = sb.tile([128, 1], i32)
        nc.gpsimd.iota(iotap_i, pattern=[[0, 1]], channel_multiplier=1)
        pm4_i = sb.tile([128, 1], i32)
        nc.vector.tensor_scalar(out=pm4_i, in0=iotap_i, scalar1=3, scalar2=None, op0=Alu.bitwise_and)
        pm4_f = sb.tile([128, 1], f32)
        nc.vector.tensor_copy(out=pm4_f, in_=pm4_i)
        pd4_i = sb.tile([128, 1], i32)
        nc.vector.tensor_scalar(out=pd4_i, in0=iotap_i, scalar1=2, scalar2=None, op0=Alu.arith_shift_right)
        pd4_f = sb.tile([128, 1], f32)
        nc.vector.tensor_copy(out=pd4_f, in_=pd4_i)
        pidx_f = sb.tile([128, 1], f32)
        nc.vector.tensor_copy(out=pidx_f, in_=iotap_i)
        iota_n_i = sb.tile([128, N], i32)
        nc.gpsimd.iota(iota_n_i, pattern=[[1, N]], channel_multiplier=0)
        iota_n = sb.tile([128, N], f32)
        nc.vector.tensor_copy(out=iota_n, in_=iota_n_i)
        # constant slot-selector: sel8[p, n] = (n == p >> 2)
        sel8c = sb.tile([128, N], f32)
        nc.vector.tensor_scalar(out=sel8c, in0=iota_n, scalar1=pd4_f[:, 0:1], scalar2=None, op0=Alu.is_equal)

        # srow[n] = matched ? gtm[n]*CPM : huge
        srow = sb.tile([1, N], f32)
        nc.vector.tensor_scalar(out=srow, in0=matched, scalar1=-OOB, scalar2=OOB, op0=Alu.mult, op1=Alu.add)
        nc.vector.scalar_tensor_tensor(out=srow, in0=gtm_r, scalar=float(CPM), in1=srow, op0=Alu.mult, op1=Alu.add)
        srow_bc = bcast(srow, N, "tP1", "srow_bc")

        sel8 = sb.tile([128, N], f32)
        nc.vector.tensor_tensor(out=sel8, in0=sel8c, in1=srow_bc[:, :], op=Alu.mult)
        gsum = sb.tile([128, 1], f32)
        nc.vector.tensor_reduce(out=gsum, in_=sel8, axis=X, op=Alu.add)
        nc.vector.tensor_tensor(out=gsum, in0=gsum, in1=pm4_f, op=Alu.add)
        tky = sb.tile([128, 1], f32)
        nc.vector.tensor_scalar(out=tky, in0=gsum, scalar1=float(VROWS), scalar2=None, op0=Alu.is_lt)
        mt = mpool.tile([128, CHUNK], f32, bufs=1)
        gidx_i = sb.tile([128, 1], i32)
        nc.vector.tensor_copy(out=gidx_i, in_=gsum)
        nc.gpsimd.indirect_dma_start(
            out=mt[:, :], out_offset=None,
            in_=masks[:, :],
            in_offset=IndirectOffsetOnAxis(ap=gidx_i[:, 0:1], axis=0),
            bounds_check=VROWS - 1, oob_is_err=False,
        )
        dd = sb.tile([128, 1], f32)
        nc.vector.tensor_scalar(out=dd, in0=pidx_f, scalar1=-OOB, scalar2=None, op0=Alu.add)
        nc.vector.tensor_tensor(out=dd, in0=dd, in1=tky, op=Alu.mult)
        nc.vector.tensor_scalar(out=dd, in0=dd, scalar1=OOB, scalar2=None, op0=Alu.add)
        didx_i = sb.tile([128, 1], i32)
        nc.vector.tensor_copy(out=didx_i, in_=dd)
        nc.gpsimd.indirect_dma_start(
            out=out_masks[:, :],
            out_offset=IndirectOffsetOnAxis(ap=didx_i[:, 0:1], axis=0),
            in_=mt[:, :], in_offset=None,
            bounds_check=OROWS - 1, oob_is_err=False,
        )

    nc.compile()
    return nc


_NC = None


def _get_nc():
    global _NC
    if _NC is None:
        _NC = build_nc()
    return _NC


def make_in_maps(pred_boxes, gt_boxes, pred_scores, pred_masks, mask_score):
    pred_boxes = np.asarray(pred_boxes, np.float32)
    gt_boxes = np.asarray(gt_boxes, np.float32)
    pred_scores = np.asarray(pred_scores, np.float32)
    pred_masks = np.asarray(pred_masks, np.float32)
    mask_score = np.asarray(mask_score, np.float32)
    in_maps = []
    for b in range(B):
        in_maps.append(
            {
                "pb": np.ascontiguousarray(pred_boxes[b]),
                "gbt": np.ascontiguousarray(gt_boxes[b].T),
                "ps": np.ascontiguousarray(pred_scores[b].reshape(M, 1)),
                "psr": np.ascontiguousarray(pred_scores[b].reshape(1, M)),
                "ms": np.ascontiguousarray(mask_score[b].reshape(M, 1)),
                "masks": np.ascontiguousarray(pred_masks[b].reshape(VROWS, CHUNK)),
            }
        )
    return in_maps


def unpack_outs(outs):
    gt_masks = np.stack([outs[b]["out_masks"].reshape(N, 256, 256) for b in range(B)])
    flags = np.stack([outs[b]["out_flags"].reshape(N) for b in range(B)])
    gtm = np.stack([outs[b]["out_gtm"].reshape(N).astype(np.int32) for b in range(B)])
    biou = np.stack([outs[b]["out_biou"].reshape(N) for b in range(B)])
    return gt_masks, flags, gtm, biou


def kernel(pred_boxes, gt_boxes, pred_scores, pred_masks, mask_score):
    from concourse.bass_utils import run_bass_kernel_spmd

    in_maps = make_in_maps(pred_boxes, gt_boxes, pred_scores, pred_masks, mask_score)
    res = run_bass_kernel_spmd(_get_nc(), in_maps, core_ids=list(range(B)))
    return unpack_outs(res.results)


# revision 27
# speedup vs baseline: 1.3604x; 1.2584x over previous
"""BoxTeacher greedy box-matching + mask gather kernel for Trainium2.

Data-parallel over the batch axis: 8 images -> 8 NeuronCores, one image per
core.  Per core:
  1. pairwise IoU [100,32] with DVE ops (gt coords broadcast across
     partitions with K=1 outer-product matmuls)
  2. active-pred filter (any iou >= 0.5) + descending-score rank compaction
     via a transposed comparison matrix and a one-hot permutation matmul
  3. sequential greedy matching, fully unrolled, wrapped in If-chunks so only
     ~n_active steps execute
  4. biou / flags via one-hot matmuls
  5. mask gather: indirect DMA HBM->SBUF->HBM moving only matched masks;
     unmatched rows use out-of-bounds indices (silently skipped) and the
     pre-zeroed output supplies the zeros.
"""

from contextlib import ExitStack

import numpy as np

import concourse.bass as bass
import concourse.bacc as bacc
import concourse.mybir as mybir
from concourse import tile
from concourse.bass import IndirectOffsetOnAxis

M = 100          # predictions per image
N = 32           # gt boxes per image
B = 8            # batch == cores
HWPROD = 256 * 256
CPM = 4          # chunks per mask
CHUNK = HWPROD // CPM   # 16384 floats per chunk row
VROWS = M * CPM  # 400 source rows
OROWS = N * CPM  # 128 dest rows
THR = 0.5
OOB = 1.0e5  # must stay positive in int32 after *CHUNK (coef) scaling
SENTINEL = 1.4999  # < 1.5 == THR+1; candidate values are 0 or >= 1.5

f32 = mybir.dt.float32
i32 = mybir.dt.int32
Alu = mybir.AluOpType
X = mybir.AxisListType.X
DVE = mybir.EngineType.DVE

USE_IF = True        # wrap scan chunks in If(n_act > c0) to skip dead steps
USE_INDIRECT = True  # gather/scatter via indirect DMA with OOB skipping
UNROLL = 16          # statically unrolled scan steps; the rest run in For_i
CHS = 4              # If-guard granularity within the unrolled steps


def build_nc():
    # Bacc's compile() splits multi-wait matmuls (HW allows one sync wait per
    # PE instruction) — raw Bass has no such pass.
    nc = bacc.Bacc(trn_type="TRN2")

    pb = nc.dram_tensor("pb", [M, 4], f32, kind="ExternalInput")
    gbt = nc.dram_tensor("gbt", [4, N], f32, kind="ExternalInput")
    ps = nc.dram_tensor("ps", [M, 1], f32, kind="ExternalInput")
    psr = nc.dram_tensor("psr", [1, M], f32, kind="ExternalInput")
    ms = nc.dram_tensor("ms", [M, 1], f32, kind="ExternalInput")
    masks = nc.dram_tensor("masks", [VROWS, CHUNK], f32, kind="ExternalInput")

    out_masks = nc.dram_tensor("out_masks", [OROWS, CHUNK], f32, kind="ExternalOutput")
    out_flags = nc.dram_tensor("out_flags", [1, N], f32, kind="ExternalOutput")
    out_gtm = nc.dram_tensor("out_gtm", [1, N], i32, kind="ExternalOutput")
    out_biou = nc.dram_tensor("out_biou", [1, N], f32, kind="ExternalOutput")

    with tile.TileContext(nc) as tc, ExitStack() as ctx:
        sb = ctx.enter_context(tc.tile_pool(name="sb", bufs=1))
        pp = ctx.enter_context(tc.tile_pool(name="pp", bufs=1, space="PSUM"))

        # ---- input loads ----
        pb_t = sb.tile([M, 4], f32)
        nc.sync.dma_start(out=pb_t, in_=pb[:, :])
        ps_t = sb.tile([M, 1], f32)
        nc.sync.dma_start(out=ps_t, in_=ps[:, :])
        psr_t = sb.tile([1, M], f32)
        nc.sync.dma_start(out=psr_t, in_=psr[:, :])
        ms_t = sb.tile([M, 1], f32)
        nc.sync.dma_start(out=ms_t, in_=ms[:, :])
        gbr = []
        for c in range(4):
            t = sb.tile([1, N], f32, name=f"gbr{c}")
            nc.sync.dma_start(out=t, in_=gbt[c : c + 1, :])
            gbr.append(t)

        ones_row = sb.tile([1, 128], f32)
        nc.vector.memset(ones_row, 1.0)
        ones_col = sb.tile([M, 1], f32)
        nc.vector.memset(ones_col, 1.0)

        iota_col_i = sb.tile([M, 1], i32)
        nc.gpsimd.iota(iota_col_i, pattern=[[0, 1]], channel_multiplier=1)
        iota_col = sb.tile([M, 1], f32)
        nc.vector.tensor_copy(out=iota_col, in_=iota_col_i)
        iota_row_i = sb.tile([M, M], i32)
        nc.gpsimd.iota(iota_row_i, pattern=[[1, M]], channel_multiplier=0)
        iota_row = sb.tile([M, M], f32)
        nc.vector.tensor_copy(out=iota_row, in_=iota_row_i)

        def bcast(row_ap, n, ptag, name):
            """Broadcast a [1, n] partition-0 row to all 128 partitions."""
            t = pp.tile([128, n], f32, tag=ptag, name=name)
            nc.tensor.matmul(out=t, lhsT=ones_row, rhs=row_ap, start=True, stop=True)
            return t

        # ---- broadcast gt coords: gbc[c][p, n] = gt_boxes[n, c] ----
        gbc = []
        for c in range(4):
            t = bcast(gbr[c], N, f"tP{c}", f"gbc{c}_ps")
            # ALU ops may read at most one PSUM operand; stage in SBUF.
            s = sb.tile([M, N], f32, name=f"gbs{c}")
            nc.vector.tensor_copy(out=s, in_=t[:M, :])
            gbc.append(s)

        # scores broadcast: sbc[p, j] = scores[j]
        sbc = pp.tile([128, M], f32, tag="tP4")
        nc.tensor.matmul(out=sbc, lhsT=ones_row, rhs=psr_t, start=True, stop=True)

        # ---- IoU [M, N] ----
        px1, py1 = pb_t[:, 0:1], pb_t[:, 1:2]
        px2, py2 = pb_t[:, 2:3], pb_t[:, 3:4]
        MN = [M, N]
        ltx = sb.tile(MN, f32)
        nc.vector.tensor_scalar(out=ltx, in0=gbc[0], scalar1=px1, scalar2=None, op0=Alu.max)
        lty = sb.tile(MN, f32)
        nc.vector.tensor_scalar(out=lty, in0=gbc[1], scalar1=py1, scalar2=None, op0=Alu.max)
        rbx = sb.tile(MN, f32)
        nc.vector.tensor_scalar(out=rbx, in0=gbc[2], scalar1=px2, scalar2=None, op0=Alu.min)
        rby = sb.tile(MN, f32)
        nc.vector.tensor_scalar(out=rby, in0=gbc[3], scalar1=py2, scalar2=None, op0=Alu.min)
        # wx = clip(rbx - ltx, 0), wy likewise
        wx = sb.tile(MN, f32)
        nc.vector.scalar_tensor_tensor(out=wx, in0=ltx, scalar=-1.0, in1=rbx, op0=Alu.mult, op1=Alu.add)
        nc.vector.tensor_scalar(out=wx, in0=wx, scalar1=0.0, scalar2=None, op0=Alu.max)
        wy = sb.tile(MN, f32)
        nc.vector.scalar_tensor_tensor(out=wy, in0=lty, scalar=-1.0, in1=rby, op0=Alu.mult, op1=Alu.add)
        nc.vector.tensor_scalar(out=wy, in0=wy, scalar1=0.0, scalar2=None, op0=Alu.max)
        inter = sb.tile(MN, f32)
        nc.vector.tensor_tensor(out=inter, in0=wx, in1=wy, op=Alu.mult)
        # gt areas broadcast
        agw = sb.tile(MN, f32)
        nc.vector.tensor_tensor(out=agw, in0=gbc[2], in1=gbc[0], op=Alu.subtract)
        agh = sb.tile(MN, f32)
        nc.vector.tensor_tensor(out=agh, in0=gbc[3], in1=gbc[1], op=Alu.subtract)
        nc.vector.tensor_tensor(out=agw, in0=agw, in1=agh, op=Alu.mult)  # agw = gt area
        # pred areas
        apw = sb.tile([M, 1], f32)
        nc.vector.tensor_tensor(out=apw, in0=px2, in1=px1, op=Alu.subtract)
        aph = sb.tile([M, 1], f32)
        nc.vector.tensor_tensor(out=aph, in0=py2, in1=py1, op=Alu.subtract)
        nc.vector.tensor_tensor(out=apw, in0=apw, in1=aph, op=Alu.mult)  # apw = pred area
        union = sb.tile(MN, f32)
        nc.vector.tensor_scalar(out=union, in0=agw, scalar1=apw[:, 0:1], scalar2=None, op0=Alu.add)
        nc.vector.scalar_tensor_tensor(out=union, in0=inter, scalar=-1.0, in1=union, op0=Alu.mult, op1=Alu.add)
        iou_t = sb.tile(MN, f32)
        nc.vector.reciprocal(out=iou_t, in_=union)
        nc.vector.tensor_tensor(out=iou_t, in0=inter, in1=iou_t, op=Alu.mult)

        # ---- active preds + ranks ----
        ok_u = sb.tile(MN, f32)
        nc.vector.tensor_scalar(out=ok_u, in0=iou_t, scalar1=THR, scalar2=None, op0=Alu.is_ge)
        act = sb.tile([M, 1], f32)
        nc.vector.tensor_reduce(out=act, in_=ok_u, axis=X, op=Alu.max)

        # before(j, i) with j on partitions, i on free:
        #   bmat[j, i] = (s[j] > s[i]) + (s[j] == s[i]) * (j < i)
        bmat = sb.tile([M, M], f32)
        nc.vector.tensor_scalar(out=bmat, in0=sbc[:M, :], scalar1=ps_t[:, 0:1], scalar2=None, op0=Alu.is_lt)
        beq = sb.tile([M, M], f32)
        nc.vector.tensor_scalar(out=beq, in0=sbc[:M, :], scalar1=ps_t[:, 0:1], scalar2=None, op0=Alu.is_equal)
        jgt = sb.tile([M, M], f32)  # [j, i] = (i > j)
        nc.vector.tensor_scalar(out=jgt, in0=iota_row, scalar1=iota_col[:, 0:1], scalar2=None, op0=Alu.is_gt)
        nc.vector.tensor_tensor(out=beq, in0=beq, in1=jgt, op=Alu.mult)
        nc.vector.tensor_tensor(out=bmat, in0=bmat, in1=beq, op=Alu.add)
        # rank among active: ranka[i] = sum_j bmat[j, i] * act[j]
        bact = sb.tile([M, M], f32)
        nc.vector.tensor_scalar(out=bact, in0=bmat, scalar1=act[:, 0:1], scalar2=None, op0=Alu.mult)
        ranka_ps = pp.tile([M, 1], f32, tag="tP5")
        nc.tensor.matmul(out=ranka_ps, lhsT=bact, rhs=ones_col[:, 0:1], start=True, stop=True)
        ranka = sb.tile([M, 1], f32)
        nc.vector.tensor_copy(out=ranka, in_=ranka_ps)
        pen = sb.tile([M, 1], f32)
        nc.vector.tensor_scalar(out=pen, in0=act, scalar1=-200.0, scalar2=200.0, op0=Alu.mult, op1=Alu.add)
        nc.vector.tensor_tensor(out=ranka, in0=ranka, in1=pen, op=Alu.add)  # rprime
        onehot = sb.tile([M, M], f32)
        nc.vector.tensor_scalar(out=onehot, in0=iota_row, scalar1=ranka[:, 0:1], scalar2=None, op0=Alu.is_equal)

        # n_active -> int32 scalar for the If cascade
        nact_ps = pp.tile([1, 1], f32, tag="tP2x")
        nc.tensor.matmul(out=nact_ps, lhsT=act[:, 0:1], rhs=ones_col[:, 0:1], start=True, stop=True)
        nact_i = sb.tile([1, 1], i32)
        nc.vector.tensor_copy(out=nact_i, in_=nact_ps)

        # sorted iou rows (score-descending among active), G = (iou+1)*(iou>=thr)
        ious = pp.tile([M, N], f32, tag="tP0")
        nc.tensor.matmul(out=ious, lhsT=onehot, rhs=iou_t, start=True, stop=True)
        okm = sb.tile([M, N], i32)  # copy_predicated masks must be integer dtype
        nc.vector.tensor_scalar(out=okm, in0=ious, scalar1=THR, scalar2=None, op0=Alu.is_ge)
        iop1 = sb.tile([M, N], f32)
        nc.vector.tensor_scalar(out=iop1, in0=ious, scalar1=1.0, scalar2=None, op0=Alu.add)
        G = sb.tile([M, N], f32)
        nc.vector.memset(G, 0.0)
        nc.vector.copy_predicated(out=G, mask=okm, data=iop1)  # NaN-safe

        orig_ps = pp.tile([1, M], f32, tag="tP1")
        nc.tensor.matmul(out=orig_ps, lhsT=iota_col[:, 0:1], rhs=onehot, start=True, stop=True)
        orig_row = sb.tile([1, M], f32)
        nc.vector.tensor_copy(out=orig_row, in_=orig_ps)

        # DVE operands need partition-aligned bases, so flatten G's rows onto
        # partition 0 for the scan's per-step row slices.  Split the flatten
        # so the first UNROLL steps can start before the long tail lands.
        g_flat = sb.tile([1, M * N], f32)
        nc.sync.dma_start(
            out=g_flat[0:1, : UNROLL * N].rearrange("p (k n) -> p k n", n=N),
            in_=G[:UNROLL, :],
        )
        nc.sync.dma_start(
            out=g_flat[0:1, UNROLL * N :].rearrange("p (k n) -> p k n", n=N),
            in_=G[UNROLL:, :],
        )

        # ---- greedy scan ----
        # skip_runtime_bounds_check: the emitted runtime assert aborts NEFF
        # execution under the PJRT path; nact is structurally within [0, M].
        nact_v = nc.values_load(
            nact_i[0:1, 0:1], engines=(DVE,), min_val=0, max_val=M,
            skip_runtime_bounds_check=True,
        )
        # For_i's back-edge barrier needs the bound on every engine.
        nact_all = nc.values_load(
            nact_i[0:1, 0:1], min_val=0, max_val=M,
            skip_runtime_bounds_check=True,
        )
        gtm_r = sb.tile([1, N], f32)
        nc.vector.memset(gtm_r, -1.0)
        cp1 = sb.tile([1, N], f32)
        mx = sb.tile([1, 1], f32)
        mxc = sb.tile([1, 1], f32)
        sel = sb.tile([1, N], i32)

        def scan_step(g_row, ik_cell):
            # cand+1 = (gtm <= 0) * G[k]
            nc.vector.scalar_tensor_tensor(
                out=cp1, in0=gtm_r, scalar=0.0, in1=g_row,
                op0=Alu.is_le, op1=Alu.mult,
            )
            nc.vector.tensor_reduce(out=mx, in_=cp1, axis=X, op=Alu.max)
            nc.vector.tensor_scalar(out=mxc, in0=mx, scalar1=SENTINEL, scalar2=None, op0=Alu.max)
            nc.vector.tensor_scalar(out=sel, in0=cp1, scalar1=mxc[0:1, 0:1], scalar2=None, op0=Alu.is_equal)
            nc.vector.copy_predicated(out=gtm_r, mask=sel, data=ik_cell.to_broadcast([1, N]))

        from contextlib import nullcontext

        # Fast path: UNROLL static steps guarded at CHS granularity (near
        # branches inside one IRAM block).  Rare overflow (n_act > UNROLL)
        # runs in a dynamic loop — slow per step, but typically 0 iterations.
        for c0 in range(0, UNROLL, CHS):
            with tc.If(nact_v > c0) if USE_IF else nullcontext():
                for k in range(c0, c0 + CHS):
                    scan_step(
                        g_flat[0:1, k * N : (k + 1) * N],
                        orig_row[0:1, k : k + 1],
                    )
        with tc.For_i(UNROLL, nact_all, 1) as kv:
            scan_step(
                g_flat[0:1, bass.ds(kv * N, N)],
                orig_row[0:1, bass.ds(kv, 1)],
            )

        # ---- outputs: gtm, matched, biou, flags ----
        gtm_i = sb.tile([1, N], i32)
        nc.vector.tensor_copy(out=gtm_i, in_=gtm_r)
        nc.sync.dma_start(out=out_gtm[:, :], in_=gtm_i)

        matched = sb.tile([1, N], f32)
        nc.vector.tensor_scalar(out=matched, in0=gtm_r, scalar1=0.0, scalar2=None, op0=Alu.is_ge)

        gtm_bc = bcast(gtm_r, N, "tP3", "gtm_bc")
        onehot2 = sb.tile([M, N], f32)
        nc.vector.tensor_scalar(out=onehot2, in0=gtm_bc[:M, :], scalar1=iota_col[:, 0:1], scalar2=None, op0=Alu.is_equal)
        onehot2_i = sb.tile([M, N], i32)
        nc.vector.tensor_scalar(out=onehot2_i, in0=gtm_bc[:M, :], scalar1=iota_col[:, 0:1], scalar2=None, op0=Alu.is_equal)
        miou = sb.tile([M, N], f32)
        nc.vector.memset(miou, 0.0)
        nc.vector.copy_predicated(out=miou, mask=onehot2_i, data=iou_t)  # NaN-safe
        biou_ps = pp.tile([1, N], f32, tag="tP4")
        nc.tensor.matmul(out=biou_ps, lhsT=ones_col[:, 0:1], rhs=miou, start=True, stop=True)
        flags_ps = pp.tile([1, N], f32, tag="tP5")
        nc.tensor.matmul(out=flags_ps, lhsT=ms_t[:, 0:1], rhs=onehot2, start=True, stop=True)
        biou_s = sb.tile([1, N], f32)
        nc.vector.tensor_copy(out=biou_s, in_=biou_ps)
        nc.sync.dma_start(out=out_biou[:, :], in_=biou_s)
        flags_s = sb.tile([1, N], f32)
        nc.vector.tensor_copy(out=flags_s, in_=flags_ps)
        nc.sync.dma_start(out=out_flags[:, :], in_=flags_s)

        # ---- mask gather: per-slot DRAM->DRAM HWDGE DMAs ----
        # srow[n] = matched ? gtm[n]*CPM : OOBROW (rows OOBROW.. fall outside
        # masks, so bounds_check="skip_entire_dma" drops the whole transfer
        # while still incrementing its semaphore; the pre-zeroed output then
        # supplies the zeros for unmatched slots).
        OOBROW = 1000.0
        srow = sb.tile([1, N], f32)
        nc.vector.tensor_scalar(out=srow, in0=matched, scalar1=-OOBROW, scalar2=OOBROW, op0=Alu.mult, op1=Alu.add)
        nc.vector.scalar_tensor_tensor(out=srow, in0=gtm_r, scalar=float(CPM), in1=srow, op0=Alu.mult, op1=Alu.add)
        srow_i = sb.tile([1, N], i32)
        nc.vector.tensor_copy(out=srow_i, in_=srow)

        # Two HWDGE rings (SP + ACT) issue interleaved slots in parallel.
        for n in range(N):
            eng = nc.sync if n % 2 == 0 else nc.scalar
            etype = mybir.EngineType.SP if n % 2 == 0 else mybir.EngineType.Activation
            # Declared max keeps the AP tracer happy; the runtime value may be
            # OOBROW (out of bounds), which the DMA's skip_entire_dma check
            # turns into a skipped transfer rather than an error.
            v = nc.values_load(
                srow_i[0:1, n : n + 1], engines=(etype,),
                min_val=0, max_val=VROWS - CPM,
                skip_runtime_bounds_check=True,
            )
            eng.dma_start(
                out=out_masks[n * CPM : (n + 1) * CPM, :],
                in_=masks[bass.ds(v, CPM), :],
                bounds_check="skip_entire_dma",
            )

    nc.compile()
    return nc


_NC = None


def _get_nc():
    global _NC
    if _NC is None:
        _NC = build_nc()
    return _NC


def make_in_maps(pred_boxes, gt_boxes, pred_scores, pred_masks, mask_score):
    pred_boxes = np.asarray(pred_boxes, np.float32)
    gt_boxes = np.asarray(gt_boxes, np.float32)
    pred_scores = np.asarray(pred_scores, np.float32)
    pred_masks = np.asarray(pred_masks, np.float32)
    mask_score = np.asarray(mask_score, np.float32)
    in_maps = []
    for b in range(B):
        in_maps.append(
            {
                "pb": np.ascontiguousarray(pred_boxes[b]),
                "gbt": np.ascontiguousarray(gt_boxes[b].T),
                "ps": np.ascontiguousarray(pred_scores[b].reshape(M, 1)),
                "psr": np.ascontiguousarray(pred_scores[b].reshape(1, M)),
                "ms": np.ascontiguousarray(mask_score[b].reshape(M, 1)),
                "masks": np.ascontiguousarray(pred_masks[b].reshape(VROWS, CHUNK)),
            }
        )
    return in_maps


def unpack_outs(outs):
    gt_masks = np.stack([outs[b]["out_masks"].reshape(N, 256, 256) for b in range(B)])
    flags = np.stack([outs[b]["out_flags"].reshape(N) for b in range(B)])
    gtm = np.stack([outs[b]["out_gtm"].reshape(N).astype(np.int32) for b in range(B)])
    biou = np.stack([outs[b]["out_biou"].reshape(N) for b in range(B)])
    return gt_masks, flags, gtm, biou


def kernel(pred_boxes, gt_boxes, pred_scores, pred_masks, mask_score):
    from concourse.bass_utils import run_bass_kernel_spmd

    in_maps = make_in_maps(pred_boxes, gt_boxes, pred_scores, pred_masks, mask_score)
    res = run_bass_kernel_spmd(_get_nc(), in_maps, core_ids=list(range(B)))
    return unpack_outs(res.results)


# revision 28
# speedup vs baseline: 1.5182x; 1.1160x over previous
"""BoxTeacher greedy box-matching + mask gather kernel for Trainium2.

Data-parallel over the batch axis: 8 images -> 8 NeuronCores, one image per
core.  Per core:
  1. pairwise IoU [100,32] with DVE ops (gt coords + scores broadcast across
     partitions with one K=1 outer-product matmul of a host-packed row)
  2. active-pred filter (any iou >= 0.5) + descending-score rank compaction
     via a transposed comparison matrix and a one-hot permutation matmul
  3. sequential greedy matching: UNROLL static steps behind If guards (only
     ~n_active execute), rare overflow handled by a dynamic For_i loop
  4. mask gather: per-slot DRAM->DRAM HWDGE DMAs with register offsets;
     unmatched slots get an out-of-bounds source row and are skipped whole
     (skip_entire_dma), the pre-zeroed output supplying their zeros
  5. biou / flags via one-hot matmuls (overlaps the mask DMAs)
"""

from contextlib import ExitStack, nullcontext

import numpy as np

import concourse.bacc as bacc
import concourse.bass as bass
import concourse.mybir as mybir
from concourse import tile

M = 100          # predictions per image
N = 32           # gt boxes per image
B = 8            # batch == cores
HWPROD = 256 * 256
CPM = 4          # chunk rows per mask
CHUNK = HWPROD // CPM   # 16384 floats per chunk row
VROWS = M * CPM  # 400 source rows
OROWS = N * CPM  # 128 dest rows
PACK = 4 * N + M  # packed broadcast row: 4 gt coords + scores
THR = 0.5
SENTINEL = 1.4999  # < 1.5 == THR+1; candidate values are 0 or >= 1.5
OOBROW = 1000.0    # source row for unmatched slots -> DMA skipped

f32 = mybir.dt.float32
i32 = mybir.dt.int32
Alu = mybir.AluOpType
X = mybir.AxisListType.X
DVE = mybir.EngineType.DVE

USE_IF = True
UNROLL = 16
CHS = 4


def build_nc():
    # Bacc's compile() splits multi-wait matmuls (HW allows one sync wait per
    # PE instruction) — raw Bass has no such pass.
    nc = bacc.Bacc(trn_type="TRN2")

    pb = nc.dram_tensor("pb", [M, 4], f32, kind="ExternalInput")
    packed = nc.dram_tensor("packed", [1, PACK], f32, kind="ExternalInput")
    ps = nc.dram_tensor("ps", [M, 1], f32, kind="ExternalInput")
    ms = nc.dram_tensor("ms", [M, 1], f32, kind="ExternalInput")
    masks = nc.dram_tensor("masks", [VROWS, CHUNK], f32, kind="ExternalInput")

    out_masks = nc.dram_tensor("out_masks", [OROWS, CHUNK], f32, kind="ExternalOutput")
    out_flags = nc.dram_tensor("out_flags", [1, N], f32, kind="ExternalOutput")
    out_gtm = nc.dram_tensor("out_gtm", [1, N], i32, kind="ExternalOutput")
    out_biou = nc.dram_tensor("out_biou", [1, N], f32, kind="ExternalOutput")

    with tile.TileContext(nc) as tc, ExitStack() as ctx:
        sb = ctx.enter_context(tc.tile_pool(name="sb", bufs=1))
        pp = ctx.enter_context(tc.tile_pool(name="pp", bufs=1, space="PSUM"))

        # ---- input loads ----
        pb_t = sb.tile([M, 4], f32)
        nc.sync.dma_start(out=pb_t, in_=pb[:, :])
        prow = sb.tile([1, PACK], f32)
        nc.sync.dma_start(out=prow, in_=packed[:, :])
        ps_t = sb.tile([M, 1], f32)
        nc.sync.dma_start(out=ps_t, in_=ps[:, :])
        ms_t = sb.tile([M, 1], f32)
        nc.sync.dma_start(out=ms_t, in_=ms[:, :])

        ones_row = sb.tile([1, 128], f32)
        nc.vector.memset(ones_row, 1.0)
        ones_col = sb.tile([M, 1], f32)
        nc.vector.memset(ones_col, 1.0)

        # iota values stay < 128, exact in f32
        iota_col = sb.tile([M, 1], f32)
        nc.gpsimd.iota(iota_col, pattern=[[0, 1]], channel_multiplier=1,
                       allow_small_or_imprecise_dtypes=True)
        iota_row = sb.tile([M, M], f32)
        nc.gpsimd.iota(iota_row, pattern=[[1, M]], channel_multiplier=0,
                       allow_small_or_imprecise_dtypes=True)

        def bcast(row_ap, n, ptag, name):
            """Broadcast a [1, n] partition-0 row to all 128 partitions."""
            t = pp.tile([128, n], f32, tag=ptag, name=name)
            nc.tensor.matmul(out=t, lhsT=ones_row, rhs=row_ap, start=True, stop=True)
            return t

        # ---- one broadcast matmul for gt coords + scores ----
        bc = bcast(prow, PACK, "tPbc", "bc_ps")
        gbc = []
        for c in range(4):
            # ALU ops may read at most one PSUM operand; stage in SBUF.
            s = sb.tile([M, N], f32, name=f"gbs{c}")
            nc.vector.tensor_copy(out=s, in_=bc[:M, c * N : (c + 1) * N])
            gbc.append(s)
        sbc = bc[:M, 4 * N : 4 * N + M]  # [p, j] = scores[j], PSUM view

        # ---- IoU [M, N] ----
        px1, py1 = pb_t[:, 0:1], pb_t[:, 1:2]
        px2, py2 = pb_t[:, 2:3], pb_t[:, 3:4]
        MN = [M, N]
        ltx = sb.tile(MN, f32)
        nc.vector.tensor_scalar(out=ltx, in0=gbc[0], scalar1=px1, scalar2=None, op0=Alu.max)
        lty = sb.tile(MN, f32)
        nc.vector.tensor_scalar(out=lty, in0=gbc[1], scalar1=py1, scalar2=None, op0=Alu.max)
        rbx = sb.tile(MN, f32)
        nc.vector.tensor_scalar(out=rbx, in0=gbc[2], scalar1=px2, scalar2=None, op0=Alu.min)
        rby = sb.tile(MN, f32)
        nc.vector.tensor_scalar(out=rby, in0=gbc[3], scalar1=py2, scalar2=None, op0=Alu.min)
        # wx = clip(rbx - ltx, 0), wy likewise
        wx = sb.tile(MN, f32)
        nc.vector.scalar_tensor_tensor(out=wx, in0=ltx, scalar=-1.0, in1=rbx, op0=Alu.mult, op1=Alu.add)
        nc.vector.tensor_scalar(out=wx, in0=wx, scalar1=0.0, scalar2=None, op0=Alu.max)
        wy = sb.tile(MN, f32)
        nc.vector.scalar_tensor_tensor(out=wy, in0=lty, scalar=-1.0, in1=rby, op0=Alu.mult, op1=Alu.add)
        nc.vector.tensor_scalar(out=wy, in0=wy, scalar1=0.0, scalar2=None, op0=Alu.max)
        inter = sb.tile(MN, f32)
        nc.vector.tensor_tensor(out=inter, in0=wx, in1=wy, op=Alu.mult)
        # gt areas broadcast
        agw = sb.tile(MN, f32)
        nc.vector.tensor_tensor(out=agw, in0=gbc[2], in1=gbc[0], op=Alu.subtract)
        agh = sb.tile(MN, f32)
        nc.vector.tensor_tensor(out=agh, in0=gbc[3], in1=gbc[1], op=Alu.subtract)
        nc.vector.tensor_tensor(out=agw, in0=agw, in1=agh, op=Alu.mult)  # agw = gt area
        # pred areas
        apw = sb.tile([M, 1], f32)
        nc.vector.tensor_tensor(out=apw, in0=px2, in1=px1, op=Alu.subtract)
        aph = sb.tile([M, 1], f32)
        nc.vector.tensor_tensor(out=aph, in0=py2, in1=py1, op=Alu.subtract)
        nc.vector.tensor_tensor(out=apw, in0=apw, in1=aph, op=Alu.mult)  # apw = pred area
        union = sb.tile(MN, f32)
        nc.vector.tensor_scalar(out=union, in0=agw, scalar1=apw[:, 0:1], scalar2=None, op0=Alu.add)
        nc.vector.scalar_tensor_tensor(out=union, in0=inter, scalar=-1.0, in1=union, op0=Alu.mult, op1=Alu.add)
        iou_t = sb.tile(MN, f32)
        nc.vector.reciprocal(out=iou_t, in_=union)
        nc.vector.tensor_tensor(out=iou_t, in0=inter, in1=iou_t, op=Alu.mult)

        # ---- active preds + ranks ----
        ok_u = sb.tile(MN, f32)
        nc.vector.tensor_scalar(out=ok_u, in0=iou_t, scalar1=THR, scalar2=None, op0=Alu.is_ge)
        act = sb.tile([M, 1], f32)
        nc.vector.tensor_reduce(out=act, in_=ok_u, axis=X, op=Alu.max)

        # before(j, i) with j on partitions, i on free:
        #   bmat[j, i] = (s[j] > s[i]) + (s[j] == s[i]) * (j < i)
        bmat = sb.tile([M, M], f32)
        nc.vector.tensor_scalar(out=bmat, in0=sbc, scalar1=ps_t[:, 0:1], scalar2=None, op0=Alu.is_lt)
        beq = sb.tile([M, M], f32)
        nc.vector.tensor_scalar(out=beq, in0=sbc, scalar1=ps_t[:, 0:1], scalar2=None, op0=Alu.is_equal)
        jgt = sb.tile([M, M], f32)  # [j, i] = (i > j)
        nc.vector.tensor_scalar(out=jgt, in0=iota_row, scalar1=iota_col[:, 0:1], scalar2=None, op0=Alu.is_gt)
        nc.vector.tensor_tensor(out=beq, in0=beq, in1=jgt, op=Alu.mult)
        nc.vector.tensor_tensor(out=bmat, in0=bmat, in1=beq, op=Alu.add)
        # rank among active: ranka[i] = sum_j bmat[j, i] * act[j]
        bact = sb.tile([M, M], f32)
        nc.vector.tensor_scalar(out=bact, in0=bmat, scalar1=act[:, 0:1], scalar2=None, op0=Alu.mult)
        ranka_ps = pp.tile([M, 1], f32, tag="tP5")
        nc.tensor.matmul(out=ranka_ps, lhsT=bact, rhs=ones_col[:, 0:1], start=True, stop=True)
        ranka = sb.tile([M, 1], f32)
        nc.vector.tensor_copy(out=ranka, in_=ranka_ps)
        pen = sb.tile([M, 1], f32)
        nc.vector.tensor_scalar(out=pen, in0=act, scalar1=-200.0, scalar2=200.0, op0=Alu.mult, op1=Alu.add)
        nc.vector.tensor_tensor(out=ranka, in0=ranka, in1=pen, op=Alu.add)  # rprime
        onehot = sb.tile([M, M], f32)
        nc.vector.tensor_scalar(out=onehot, in0=iota_row, scalar1=ranka[:, 0:1], scalar2=None, op0=Alu.is_equal)

        # n_active -> int32 scalar for the If cascade
        nact_ps = pp.tile([1, 1], f32, tag="tP2x")
        nc.tensor.matmul(out=nact_ps, lhsT=act[:, 0:1], rhs=ones_col[:, 0:1], start=True, stop=True)
        nact_i = sb.tile([1, 1], i32)
        nc.vector.tensor_copy(out=nact_i, in_=nact_ps)

        # sorted iou rows (score-descending among active), G = (iou+1)*(iou>=thr)
        ious = pp.tile([M, N], f32, tag="tP0")
        nc.tensor.matmul(out=ious, lhsT=onehot, rhs=iou_t, start=True, stop=True)
        okm = sb.tile([M, N], i32)  # copy_predicated masks must be integer dtype
        nc.vector.tensor_scalar(out=okm, in0=ious, scalar1=THR, scalar2=None, op0=Alu.is_ge)
        iop1 = sb.tile([M, N], f32)
        nc.vector.tensor_scalar(out=iop1, in0=ious, scalar1=1.0, scalar2=None, op0=Alu.add)
        G = sb.tile([M, N], f32)
        nc.vector.memset(G, 0.0)
        nc.vector.copy_predicated(out=G, mask=okm, data=iop1)  # NaN-safe

        orig_ps = pp.tile([1, M], f32, tag="tP1")
        nc.tensor.matmul(out=orig_ps, lhsT=iota_col[:, 0:1], rhs=onehot, start=True, stop=True)
        orig_row = sb.tile([1, M], f32)
        nc.vector.tensor_copy(out=orig_row, in_=orig_ps)

        # DVE operands need partition-aligned bases, so flatten G's rows onto
        # partition 0 for the scan's per-step row slices.  Split the flatten
        # so the first UNROLL steps can start before the long tail lands.
        g_flat = sb.tile([1, M * N], f32)
        nc.sync.dma_start(
            out=g_flat[0:1, : UNROLL * N].rearrange("p (k n) -> p k n", n=N),
            in_=G[:UNROLL, :],
        )
        nc.sync.dma_start(
            out=g_flat[0:1, UNROLL * N :].rearrange("p (k n) -> p k n", n=N),
            in_=G[UNROLL:, :],
        )

        # ---- greedy scan ----
        # skip_runtime_bounds_check: the emitted runtime assert aborts NEFF
        # execution under the PJRT path; nact is structurally within [0, M].
        nact_v = nc.values_load(
            nact_i[0:1, 0:1], engines=(DVE,), min_val=0, max_val=M,
            skip_runtime_bounds_check=True,
        )
        # For_i's back-edge barrier needs the bound on every engine.
        nact_all = nc.values_load(
            nact_i[0:1, 0:1], min_val=0, max_val=M,
            skip_runtime_bounds_check=True,
        )
        gtm_r = sb.tile([1, N], f32)
        nc.vector.memset(gtm_r, -1.0)
        # cp1 has one extra slot pinned at SENTINEL so the row-max comes out
        # pre-clamped (saves a max op per step).
        cp1 = sb.tile([1, N + 1], f32)
        nc.vector.memset(cp1, SENTINEL)
        mxc = sb.tile([1, 1], f32)
        sel = sb.tile([1, N], i32)

        def scan_step(g_row, ik_cell):
            # cand+1 = (gtm <= 0) * G[k]
            nc.vector.scalar_tensor_tensor(
                out=cp1[0:1, :N], in0=gtm_r, scalar=0.0, in1=g_row,
                op0=Alu.is_le, op1=Alu.mult,
            )
            nc.vector.tensor_reduce(out=mxc, in_=cp1[0:1, :], axis=X, op=Alu.max)
            nc.vector.tensor_scalar(out=sel, in0=cp1[0:1, :N], scalar1=mxc[0:1, 0:1], scalar2=None, op0=Alu.is_equal)
            nc.vector.copy_predicated(out=gtm_r, mask=sel, data=ik_cell.to_broadcast([1, N]))

        # Fast path: UNROLL static steps guarded at CHS granularity (near
        # branches inside one IRAM block).  Rare overflow (n_act > UNROLL)
        # runs in a dynamic loop — slow per step, but typically 0 iterations.
        for c0 in range(0, UNROLL, CHS):
            with tc.If(nact_v > c0) if USE_IF else nullcontext():
                for k in range(c0, c0 + CHS):
                    scan_step(
                        g_flat[0:1, k * N : (k + 1) * N],
                        orig_row[0:1, k : k + 1],
                    )
        with tc.For_i(UNROLL, nact_all, 1) as kv:
            scan_step(
                g_flat[0:1, bass.ds(kv * N, N)],
                orig_row[0:1, bass.ds(kv, 1)],
            )

        # ---- mask gather first: per-slot DRAM->DRAM HWDGE DMAs ----
        matched = sb.tile([1, N], f32)
        nc.vector.tensor_scalar(out=matched, in0=gtm_r, scalar1=0.0, scalar2=None, op0=Alu.is_ge)
        # srow[n] = matched ? gtm[n]*CPM : OOBROW (rows OOBROW.. fall outside
        # masks, so bounds_check="skip_entire_dma" drops the whole transfer
        # while still incrementing its semaphore; the pre-zeroed output then
        # supplies the zeros for unmatched slots).
        srow = sb.tile([1, N], f32)
        nc.vector.tensor_scalar(out=srow, in0=matched, scalar1=-OOBROW, scalar2=OOBROW, op0=Alu.mult, op1=Alu.add)
        nc.vector.scalar_tensor_tensor(out=srow, in0=gtm_r, scalar=float(CPM), in1=srow, op0=Alu.mult, op1=Alu.add)
        srow_i = sb.tile([1, N], i32)
        nc.vector.tensor_copy(out=srow_i, in_=srow)

        # Two HWDGE rings (SP + ACT) issue interleaved slots in parallel.
        for n in range(N):
            eng = nc.sync if n % 2 == 0 else nc.scalar
            etype = mybir.EngineType.SP if n % 2 == 0 else mybir.EngineType.Activation
            # Declared max keeps the AP tracer happy; the runtime value may be
            # OOBROW (out of bounds), which the DMA's skip_entire_dma check
            # turns into a skipped transfer rather than an error.
            v = nc.values_load(
                srow_i[0:1, n : n + 1], engines=(etype,),
                min_val=0, max_val=VROWS - CPM,
                skip_runtime_bounds_check=True,
            )
            eng.dma_start(
                out=out_masks[n * CPM : (n + 1) * CPM, :],
                in_=masks[bass.ds(v, CPM), :],
                bounds_check="skip_entire_dma",
            )

        # ---- remaining outputs: gtm, biou, flags (overlap the mask DMAs) ----
        gtm_i = sb.tile([1, N], i32)
        nc.vector.tensor_copy(out=gtm_i, in_=gtm_r)
        nc.sync.dma_start(out=out_gtm[:, :], in_=gtm_i)

        gtm_bc = bcast(gtm_r, N, "tP3", "gtm_bc")
        onehot2 = sb.tile([M, N], f32)
        nc.vector.tensor_scalar(out=onehot2, in0=gtm_bc[:M, :], scalar1=iota_col[:, 0:1], scalar2=None, op0=Alu.is_equal)
        onehot2_i = sb.tile([M, N], i32)
        nc.vector.tensor_scalar(out=onehot2_i, in0=gtm_bc[:M, :], scalar1=iota_col[:, 0:1], scalar2=None, op0=Alu.is_equal)
        miou = sb.tile([M, N], f32)
        nc.vector.memset(miou, 0.0)
        nc.vector.copy_predicated(out=miou, mask=onehot2_i, data=iou_t)  # NaN-safe
        biou_ps = pp.tile([1, N], f32, tag="tP4")
        nc.tensor.matmul(out=biou_ps, lhsT=ones_col[:, 0:1], rhs=miou, start=True, stop=True)
        flags_ps = pp.tile([1, N], f32, tag="tP5")
        nc.tensor.matmul(out=flags_ps, lhsT=ms_t[:, 0:1], rhs=onehot2, start=True, stop=True)
        biou_s = sb.tile([1, N], f32)
        nc.vector.tensor_copy(out=biou_s, in_=biou_ps)
        nc.sync.dma_start(out=out_biou[:, :], in_=biou_s)
        flags_s = sb.tile([1, N], f32)
        nc.vector.tensor_copy(out=flags_s, in_=flags_ps)
        nc.sync.dma_start(out=out_flags[:, :], in_=flags_s)

    nc.compile()
    return nc


_NC = None


def _get_nc():
    global _NC
    if _NC is None:
        _NC = build_nc()
    return _NC


def make_in_maps(pred_boxes, gt_boxes, pred_scores, pred_masks, mask_score):
    pred_boxes = np.asarray(pred_boxes, np.float32)
    gt_boxes = np.asarray(gt_boxes, np.float32)
    pred_scores = np.asarray(pred_scores, np.float32)
    pred_masks = np.asarray(pred_masks, np.float32)
    mask_score = np.asarray(mask_score, np.float32)
    in_maps = []
    for b in range(B):
        packed = np.concatenate(
            [gt_boxes[b].T.reshape(-1), pred_scores[b].reshape(-1)]
        ).reshape(1, PACK)
        in_maps.append(
            {
                "pb": np.ascontiguousarray(pred_boxes[b]),
                "packed": np.ascontiguousarray(packed),
                "ps": np.ascontiguousarray(pred_scores[b].reshape(M, 1)),
                "ms": np.ascontiguousarray(mask_score[b].reshape(M, 1)),
                "masks": np.ascontiguousarray(pred_masks[b].reshape(VROWS, CHUNK)),
            }
        )
    return in_maps


def unpack_outs(outs):
    gt_masks = np.stack([outs[b]["out_masks"].reshape(N, 256, 256) for b in range(B)])
    flags = np.stack([outs[b]["out_flags"].reshape(N) for b in range(B)])
    gtm = np.stack([outs[b]["out_gtm"].reshape(N).astype(np.int32) for b in range(B)])
    biou = np.stack([outs[b]["out_biou"].reshape(N) for b in range(B)])
    return gt_masks, flags, gtm, biou


def kernel(pred_boxes, gt_boxes, pred_scores, pred_masks, mask_score):
    from concourse.bass_utils import run_bass_kernel_spmd

    in_maps = make_in_maps(pred_boxes, gt_boxes, pred_scores, pred_masks, mask_score)
    res = run_bass_kernel_spmd(_get_nc(), in_maps, core_ids=list(range(B)))
    return unpack_outs(res.results)
